# revision 30
# baseline (speedup 1.0000x reference)
"""
Binary Conv2d (BBCU-style) block on 8 Trainium2 NeuronCores.

Computation (per reference):
    z  = sign(x + move0_bias)                    # binarized activations
    bw = scale[o] * sign(W)                      # binarized weights
    y  = conv3x3(z, bw, pad=1)
    y  = prelu(y + pr_bias0, a) + pr_bias1 + x   # RPReLU + identity

This kernel is memory-roofline oriented: all large HBM I/O is fp16.

Host-side staging:
  - xh = fp16(x + pr_bias1), with a one-ulp "sign-preserving nudge" on the
    ~1e-5 fraction of elements where fp16 rounding would flip
    sign(x + move0_bias): afterwards (f32(xh) > t) == (x + move0_bias > 0)
    exactly, with t = pr_bias1 - move0_bias per channel. Folding pr_bias1
    into x lets the final epilogue be a plain tensor+tensor add, and folding
    move0_bias into the threshold makes the device sign op a single compare.
  - x is repacked to the "parity layout" the device uses: partition
    p = parity*64 + channel, free dim = (row//2)*W + col. Every DMA line is
    then fully contiguous in DRAM (2 KB+ runs instead of 1 KB strided).
  - Output y is written fp16 in the same layout; host unpacks to f32.
    Verified end-to-end rel err ~6e-4 (gate 2e-2).

Device pipeline (per core: 2 images, chunks of G=32 rows):
  z = ((xh > t) - 0.5) in {-0.5, +0.5} as fp8e4   (DVE tensor_scalar)
  conv = 3 DoubleRow fp8 matmul pairs per [128,512] PSUM tile:
    - zz plane 0 ("zs1"): slot j holds rows (2(j-1), 2(j-1)+1) on the
      (even, odd) partition blocks; byte c = col c-1; 272 B slots.
    - zz plane 1 ("zsw"): parity-swapped/shifted copy built by two
      SBUF->SBUF DMAs: at slot u, parts 0-63 = odd row 2u-3, parts
      64-127 = even row 2u. This makes the cross-pair (halo) taps of the
      3x3 conv read the SAME slot index as the in-pair taps, so each
      kw-tap is ONE DoubleRow matmul with k-tile dim = (plane0, plane1):
      plane0 lhsT = in-pair taps (kh quadrants), plane1 lhsT = halo taps
      (diagonal blocks), i.e. 6 matmul instructions per tile instead of 9,
      at 2 fp8 MACs/cell/cycle.
  epilogue: ACT Prelu(2*scale*S + pb0, alpha) -> fp16, then one DVE
  tensor+tensor add of the identity, then fp16 DMA out.
"""

import os
from contextlib import ExitStack

import numpy as np

import ml_dtypes

import concourse.bass as bass
import concourse.mybir as mybir
import concourse.tile as tile
from concourse.bass_utils import run_bass_kernel_spmd

# ---------------------------------------------------------------------------
# Workaround: the in-container walrus rejects instructions carrying more than
# 2 semaphore waits ("Too many sync wait commands" in setupSyncWait), but
# Tile's sem-assignment freely attaches 3+. Post-process the serialized BIR:
# move excess waits onto NoOp instructions inserted just before the carrier
# (same engine => program order preserves the happens-before).
# ---------------------------------------------------------------------------
_MAX_WAITS = 1


def _split_sync_waits(mod: dict, max_waits: int = _MAX_WAITS) -> dict:
    for fn in mod.get("functions", []):
        for bb in fn.get("blocks", []):
            out = []
            for ins in bb.get("instructions", []):
                si = ins.get("sync_info")
                waits = (si or {}).get("on_wait") or []
                if len(waits) > max_waits:
                    extra, keep = waits[:-max_waits], waits[-max_waits:]
                    for i in range(0, len(extra), max_waits):
                        out.append({
                            "debug": ins.get("debug", 0),
                            "engine": ins["engine"],
                            "ins": [],
                            "name": f"{ins['name']}_ws{i}",
                            "opcode": "NoOp",
                            "outs": [],
                            "sync_info": {
                                "on_update": [],
                                "on_wait": extra[i:i + max_waits],
                            },
                        })
                    si["on_wait"] = keep
                out.append(ins)
            bb["instructions"] = out
    return mod


# ---------------------------------------------------------------------------
# Tile emits one Ldweights per Matmult, but our inner loop issues runs of 8
# matmuls sharing the same stationary operand; the redundant ~150 ns weight
# reloads gate the PE. Drop an Ldweights identical to the previous one on the
# PE stream (weights stay stationary across the intervening non-self-loading
# Matmults). One with semaphore waits becomes a NoOp carrying them instead.
# ---------------------------------------------------------------------------
_LDW_KEYS = ("ins", "perf_mode", "tile_position", "tile_size")


def _dedupe_ldweights(mod: dict) -> dict:
    for fn in mod.get("functions", []):
        for bb in fn.get("blocks", []):
            out = []
            last = None
            for ins in bb.get("instructions", []):
                if ins.get("engine") != "PE":
                    out.append(ins)
                    continue
                if ins["opcode"] == "Ldweights":
                    key = repr([ins.get(k) for k in _LDW_KEYS])
                    if key == last:
                        si = ins.get("sync_info")
                        if si and (si.get("on_wait") or si.get("on_update")):
                            out.append({
                                "debug": ins.get("debug", 0),
                                "engine": "PE",
                                "ins": [],
                                "name": f"{ins['name']}_ldwdup",
                                "opcode": "NoOp",
                                "outs": [],
                                "sync_info": si,
                            })
                        continue
                    last = key
                elif ins["opcode"] != "Matmult" or ins.get("ldweights"):
                    last = None
                out.append(ins)
            bb["instructions"] = out
    return mod


_orig_to_json_bytes = bass.Bass.to_json_bytes


def _to_json_bytes_split(self):
    import orjson

    mod = orjson.loads(_orig_to_json_bytes(self))
    if os.environ.get("BBCU_LDW_DEDUPE", "1") == "1":
        mod = _dedupe_ldweights(mod)
    return orjson.dumps(_split_sync_waits(mod))


bass.Bass.to_json_bytes = _to_json_bytes_split

F32 = mybir.dt.float32
F16 = mybir.dt.float16
FP8 = mybir.dt.float8e4
NP_FP8 = ml_dtypes.float8_e4m3

# consts column indices
C_T = 0       # sign threshold  t = pr_bias1 - move0_bias
C_SC = 1      # 2 * scale  (z is +-0.5)
C_PB0 = 2     # pr_bias0
C_AL = 3      # prelu alpha
NCOL = 4

SLOT = 272    # bytes per row-pair slot (16-aligned, >= 258)
NCORES = 8


def _build(Bc: int, H: int, W: int, C: int, G: int, use_prelu: bool = True):
    """Per-core Bass module: inputs x [Bc,128,(H/2)*W] f16 (parity layout),
    wp [128, 3*2*128] fp8, cv [128,NCOL] f32; output y same layout as x."""
    assert C == 64 and W == 256
    assert H % G == 0 and G % 4 == 0
    P = G // 2            # row-pairs per chunk
    NCH = H // G          # chunks per image
    NPAIR = H // 2
    NSLOT = 66  # ring positions per plane (RING + zero-lo + top)
    FREE = P * W          # free elems per chunk

    dump_zz = os.environ.get("BBCU_DUMP_ZZ", "0") == "1"
    nc = bass.Bass()
    xd = nc.declare_dram_parameter("x", [Bc, 128, NPAIR * W], F16, isOutput=False)
    wd = nc.declare_dram_parameter("wp", [128, 3 * 2 * 128], FP8, isOutput=False)
    cd = nc.declare_dram_parameter("cv", [128, NCOL], F32, isOutput=False)
    yd = nc.declare_dram_parameter("y", [Bc, 128, NPAIR * W], F16, isOutput=True)
    zzd = (nc.declare_dram_parameter("zzd", [128, 2 * NSLOT * SLOT], FP8,
                                     isOutput=True) if dump_zz else None)

    with ExitStack() as ctx:
        tc = ctx.enter_context(tile.TileContext(nc))
        cpool = ctx.enter_context(tc.tile_pool(name="const", bufs=1))
        zpool = ctx.enter_context(tc.tile_pool(name="zz", bufs=1))
        xpool = ctx.enter_context(tc.tile_pool(name="xt", bufs=5))
        gpool = ctx.enter_context(tc.tile_pool(name="gt", bufs=5))
        pspool = ctx.enter_context(tc.tile_pool(name="ps", bufs=4, space="PSUM"))

        wsb = cpool.tile([128, 3 * 2 * 128], FP8)
        nc.sync.dma_start(wsb[:], wd[:])
        cvs = cpool.tile([128, NCOL], F32)
        nc.sync.dma_start(cvs[:], cd[:])

        # z storage: two planes (zs1 / zsw), each a ring of RING data slots
        # plus a dedicated zero slot (pos 0, rows below the image) and a
        # dedicated top slot (pos RING+1, global slot NPAIR). Slots within a
        # chunk land contiguously in the ring, so the zsw copies move
        # contiguous multi-KB runs, and the plane stride (NPOS*SLOT) fits the
        # 16-bit ISA step field of the DoubleRow k-tile dim.
        RING = 64
        NPOS = RING + 2
        PSTRIDE = NPOS * SLOT

        def pos(s):
            if s == 0:
                return 0
            if s == NPAIR:
                return RING + 1
            return (s - 1) % RING + 1

        zz = zpool.tile([128, 2 * PSTRIDE], FP8)
        zzpair = zz[:].rearrange("p (t r) -> p t r", t=2)
        zzv0 = zz[:, 0:PSTRIDE].rearrange("p (s c) -> p s c", c=SLOT)
        zzv1 = zz[:, PSTRIDE:2 * PSTRIDE].rearrange("p (s c) -> p s c", c=SLOT)

        # one-time pads:
        # plane0 col pads (col -1 at byte 0, col 256.. at bytes 257+); plane1
        # pads are copied from plane0 by the zsw DMAs.
        nc.gpsimd.memset(zzv0[:, :, 0:1], 0.0)
        nc.gpsimd.memset(zzv0[:, :, 1 + W:SLOT], 0.0)
        # plane0 rows-below-image halo: pos 0
        nc.gpsimd.memset(zzv0[:, 0:1, :], 0.0)
        # plane1 top slot, parts 64-127 = row H (below image): zero
        nc.gpsimd.memset(zzv1[64:128, RING + 1:RING + 2, :], 0.0)

        def runs(pairs):
            """Group (dst_pos, src_pos) pairs into runs consecutive in both."""
            out = []
            for d, s in pairs:
                if out and out[-1][0] + out[-1][2] == d and \
                        out[-1][1] + out[-1][2] == s:
                    out[-1][2] += 1
                else:
                    out.append([d, s, 1])
            return out

        # weight APs: wp cols = [kw(3), ktile(2), m(128)]
        w_aps = [
            wsb[:, kw * 256:(kw + 1) * 256].rearrange("k (t m) -> k t m", t=2)
            for kw in range(3)
        ]

        def load(b, k):
            xt = xpool.tile([128, FREE], F16, name=f"xt_{b}_{k}", tag="xt")
            nc.sync.dma_start(xt[:], xd[b, :, k * FREE:(k + 1) * FREE])
            return xt

        def sign(b, k, xt):
            s0 = k * P + 1
            xtv = xt[:].rearrange("p (s c) -> p s c", c=W)
            for d, s, n in runs([(pos(s0 + i), i) for i in range(P)]):
                nc.vector.tensor_scalar(
                    zzv0[:, d:d + n, 1:1 + W],
                    xtv[:, s:s + n, :],
                    cvs[:, C_T:C_T + 1],
                    0.5,
                    mybir.AluOpType.is_gt,
                    mybir.AluOpType.subtract,
                )

        def zsw(b, k):
            s0 = k * P + 1
            # plane1 parts 0-63 slot u <- plane0 parts 64-127 slot u-1
            # (A on the sync queue, B on gpsimd: spreads descriptor-issue cost)
            for d, s, n in runs([(pos(s0 + i), pos(s0 + i - 1))
                                 for i in range(P)]):
                nc.sync.dma_start(
                    zzv1[0:64, d:d + n, :], zzv0[64:128, s:s + n, :])
            # plane1 parts 64-127 slot u <- plane0 parts 0-63 slot u+1
            u0 = max(k * P, 1)
            for d, s, n in runs([(pos(u), pos(u + 1))
                                 for u in range(u0, k * P + P)]):
                nc.gpsimd.dma_start(
                    zzv1[64:128, d:d + n, :], zzv0[0:64, s:s + n, :])

        def conv(b, k, xt):
            gt = gpool.tile([128, FREE], F16, name=f"gt_{b}_{k}", tag="gt")
            NT = P // 2
            for grp in range(0, NT, 4):
                tiles = range(grp, min(grp + 4, NT))
                # pairs of tiles share a 2-bank psum tile so each prelu
                # drains 1024 cols in one instruction (halves ACT overhead)
                ps2 = {t: pspool.tile([128, 1024], F32, name="ps2")
                       for t in tiles if t % 2 == 0}
                # kw outer so the stationary weights reload only 3x per group.
                # start=True clears the whole bank's has_written bits, so it
                # must appear exactly once per bank (first MM), stop on the
                # last; the per-element has_written handles the two halves.
                for kw in range(3):
                    for t in tiles:
                        s = k * P + 2 * t + 1
                        for sl in range(2):
                            q = pos(s + sl)
                            off = (t % 2) * 512 + sl * 256
                            nc.tensor.matmul(
                                ps2[t - t % 2][:, off:off + 256],
                                w_aps[kw],
                                zzpair[:, :, q * SLOT + kw:q * SLOT + kw + 256],
                                start=(kw == 0 and sl == 0),
                                stop=(kw == 2 and sl == 1),
                                skip_group_check=True,
                                perf_mode=mybir.MatmulPerfMode.DoubleRow,
                            )
                for t in tiles:
                    if t % 2:
                        continue
                    nc.scalar.activation(
                        gt[:, t * 512:(t + 2) * 512],
                        ps2[t][:],
                        mybir.ActivationFunctionType.Prelu,
                        bias=cvs[:, C_PB0:C_PB0 + 1],
                        scale=cvs[:, C_SC:C_SC + 1],
                        alpha=cvs[:, C_AL:C_AL + 1],
                    )
            # out = g + xh  (identity + pr_bias1, pre-folded on host),
            # split across DVE and GPSIMD in proportion to their measured
            # elementwise rates (~1.1 vs ~2.0 ns per 128-lane column).
            FD = (FREE * 3 // 4) // 256 * 256
            nc.vector.scalar_tensor_tensor(
                gt[:, 0:FD], gt[:, 0:FD], 0.0, xt[:, 0:FD],
                op0=mybir.AluOpType.add, op1=mybir.AluOpType.add)
            nc.gpsimd.tensor_tensor(
                gt[:, FD:FREE], gt[:, FD:FREE], xt[:, FD:FREE],
                mybir.AluOpType.add)
            nc.sync.dma_start(yd[b, :, k * FREE:(k + 1) * FREE], gt[:])

        # Pipeline: loads run 1 job ahead; the output store is emitted 2 jobs
        # late so that by the time it reaches the head of the sync engine's
        # FIFO its producers (prelu + identity add) have long finished —
        # otherwise it blocks the next input loads behind it (head-of-line)
        # and starves the PE.
        # Software pipeline: loads lead by 1 job; conv(k) trails sign/zsw(k+1)
        # by one emission step (its last tile needs the k+1 halo row).
        jobs = [(b, k) for b in range(Bc) for k in range(NCH)]
        xts = {}
        xts[jobs[0]] = load(*jobs[0])
        for idx, (b, k) in enumerate(jobs):
            if idx + 1 < len(jobs):
                xts[jobs[idx + 1]] = load(*jobs[idx + 1])
            sign(b, k, xts[(b, k)])
            zsw(b, k)
            if idx >= 1:
                bb, kk = jobs[idx - 1]
                conv(bb, kk, xts.pop((bb, kk)))
        conv(*jobs[-1], xts.pop(jobs[-1]))
        if dump_zz:
            nc.sync.dma_start(zzd[:], zz[:])

    return nc


def _host_prep(x, move0_bias, conv_weight, prelu_weight, pr_bias0, pr_bias1,
               n_cores=NCORES):
    """Returns (xh_packed [n_cores][Bc,128,(H/2)*W] f16, wp fp8, cv f32)."""
    B, C, H, W = x.shape
    b0 = np.asarray(move0_bias, np.float32).reshape(C)
    pb1 = np.asarray(pr_bias1, np.float32).reshape(C)
    t = (pb1 - b0).astype(np.float32)

    # fp16(x + pb1) with sign-preserving nudge
    xb = x + pb1.reshape(1, C, 1, 1)
    xh = xb.astype(np.float16)
    tb = t.reshape(1, C, 1, 1)
    sref = (x + b0.reshape(1, C, 1, 1)) > 0
    pdev = xh.astype(np.float32) > tb
    fixup = sref & ~pdev
    fixdn = ~sref & pdev
    if fixup.any():
        xh = np.where(fixup, np.nextafter(xh, np.float16(np.inf),
                                          dtype=np.float16), xh)
    if fixdn.any():
        xh = np.where(fixdn, np.nextafter(xh, np.float16(-np.inf),
                                          dtype=np.float16), xh)

    # parity repack: [B,C,H,W] -> [B, par, C, H/2, W] -> [B, 128, (H/2)*W]
    xp = np.ascontiguousarray(
        xh.reshape(B, C, H // 2, 2, W).transpose(0, 3, 1, 2, 4)
    ).reshape(B, 128, (H // 2) * W)

    # weights: wp[k, kw*256 + tile*128 + m] ; k = par_k*64+ci, m = par_m*64+co
    w = np.asarray(conv_weight, np.float32)
    sw = np.sign(w).astype(np.float32)            # [co, ci, kh, kw]
    swT = np.transpose(sw, (1, 0, 2, 3))          # [ci, co, kh, kw]
    scale = np.mean(np.abs(w), axis=(1, 2, 3)).astype(np.float32)
    wp = np.zeros((128, 3, 2, 128), dtype=np.float32)
    for kw in range(3):
        # plane 0: in-pair taps
        wp[0:64, kw, 0, 0:64] = swT[:, :, 1, kw]      # even->even  kh=1
        wp[0:64, kw, 0, 64:128] = swT[:, :, 0, kw]    # even->odd   kh=0
        wp[64:128, kw, 0, 0:64] = swT[:, :, 2, kw]    # odd->even   kh=2
        wp[64:128, kw, 0, 64:128] = swT[:, :, 1, kw]  # odd->odd    kh=1
        # plane 1: halo taps via zsw (diagonal blocks)
        wp[0:64, kw, 1, 0:64] = swT[:, :, 0, kw]      # row 2p-1 -> even out
        wp[64:128, kw, 1, 64:128] = swT[:, :, 2, kw]  # row 2p+2 -> odd out
    wp8 = wp.reshape(128, 3 * 2 * 128).astype(NP_FP8)

    a = np.asarray(prelu_weight, np.float32).reshape(C)
    pb0 = np.asarray(pr_bias0, np.float32).reshape(C)
    cv = np.zeros((128, NCOL), dtype=np.float32)
    for blk in range(2):
        s = slice(blk * 64, blk * 64 + 64)
        cv[s, C_T] = t
        cv[s, C_SC] = 2.0 * scale
        cv[s, C_PB0] = pb0
        cv[s, C_AL] = a
    return xp, wp8, cv


def _unpack(y_packed, B, C, H, W):
    """[B,128,(H/2)*W] f16 -> [B,C,H,W] f32"""
    y = y_packed.reshape(B, 2, C, H // 2, W).transpose(0, 2, 3, 1, 4)
    return np.ascontiguousarray(y).reshape(B, C, H, W).astype(np.float32)


_NC_CACHE: dict = {}


def _get_nc(key, *args, **kw):
    if key not in _NC_CACHE:
        _NC_CACHE[key] = _build(*args, **kw)
    return _NC_CACHE[key]


def prepare(x, move0_bias, conv_weight, prelu_weight, pr_bias0, pr_bias1):
    x = np.asarray(x, dtype=np.float32)
    B, C, H, W = x.shape
    assert B % NCORES == 0
    Bc = B // NCORES
    G = 32
    use_prelu = os.environ.get("BBCU_NO_PRELU", "0") != "1"
    xp, wp8, cv = _host_prep(x, move0_bias, conv_weight, prelu_weight,
                             pr_bias0, pr_bias1)
    key = (Bc, H, W, C, G, use_prelu)
    nc = _get_nc(key, Bc, H, W, C, G, use_prelu)
    in_maps = [
        {"x": xp[i * Bc:(i + 1) * Bc], "wp": wp8, "cv": cv}
        for i in range(NCORES)
    ]
    return nc, in_maps, (B, C, H, W, Bc)


def kernel(x, move0_bias, conv_weight, prelu_weight, pr_bias0, pr_bias1):
    nc, in_maps, (B, C, H, W, Bc) = prepare(
        x, move0_bias, conv_weight, prelu_weight, pr_bias0, pr_bias1)
    res = run_bass_kernel_spmd(nc, in_maps, core_ids=list(range(NCORES)))
    yp = np.concatenate([res.results[i]["y"] for i in range(NCORES)], axis=0)
    return _unpack(yp, B, C, H, W)
